# revision 1
# baseline (speedup 1.0000x reference)
"""Trainium2 Bass kernel for the masked style-attention module.

Shapes (hardcoded): B=4, C_IN=256, C_KEY=448, H=W=64, N=4096.
Sharding: 8 cores = batch (4) x query-row half (2). Each core computes
out[b][:, h*2048:(h+1)*2048] for its (b, h).

Math per core (rows n in its half, all m in 0..4095):
  Fq[c,n] = Wf@ckey + bf      (bias via appended ones-row in contraction)
  G [c,m] = Wg@skey + bg
  Hv[m,c] = (Wh@style + bh)^T (computed directly transposed: lhsT=style)
  S [n,m] = sum_c Fq[c,n] G[c,m] + cm_neg[n]*smi[m]   (mask folded in as an
            extra contraction row; additive -1e15 == masked_fill in fp32)
  P = exp(S)  (no row-max pass: |S| < ~40 so exp never overflows; softmax
            is shift-invariant so result matches the reference)
  mean = (P @ Hv) / rowsum ; m2 = (P @ Hv^2) / rowsum
  out[c,n] = sqrt(relu(m2-mean^2))[n,c]^T * mvn(content)[c,n] + mean[n,c]^T
"""

from contextlib import ExitStack

import numpy as np

import concourse.bass as bass
from concourse import bacc
import concourse.mybir as mybir
import concourse.tile as tile
from concourse.bass_utils import run_bass_kernel_spmd

AF = mybir.ActivationFunctionType
ALU = mybir.AluOpType
AX = mybir.AxisListType
F32 = mybir.dt.float32
F32R = mybir.dt.float32r

B, C_IN, C_KEY = 4, 256, 448
N = 4096
HALF = 2048
NGRP = 4          # groups of 4 blocks (512 query rows each)
NEG = -1e15
EPS = 1e-5
CORR = N / (N - 1.0)  # unbiased-variance correction for mvn

# contraction tiles over 449 (= C_KEY + bias/mask row)
KT449 = [(0, 128), (128, 128), (256, 128), (384, 65)]
# contraction tiles over 257 (= C_IN + bias row)
KT257 = [(0, 128), (128, 128), (256, 1)]
# output-channel tiles over 448
CO448 = [(0, 128), (128, 128), (256, 128), (384, 64)]


def _r(ap):
    return ap if ap.dtype == F32R else ap.bitcast(F32R)


def _build():
    nc = bacc.Bacc("TRN2", target_bir_lowering=False)

    skey = nc.dram_tensor("skey", [449, N], F32R, kind="ExternalInput")
    wgT = nc.dram_tensor("wgT", [449, 448], F32R, kind="ExternalInput")
    ckey = nc.dram_tensor("ckey", [449, HALF], F32R, kind="ExternalInput")
    wfT = nc.dram_tensor("wfT", [449, 448], F32R, kind="ExternalInput")
    styl = nc.dram_tensor("styl", [256, N], F32R, kind="ExternalInput")
    whT = nc.dram_tensor("whT", [256, 256], F32R, kind="ExternalInput")
    bhr = nc.dram_tensor("bhr", [1, 256], F32, kind="ExternalInput")
    cont = nc.dram_tensor("cont", [256, N], F32, kind="ExternalInput")
    smi = nc.dram_tensor("smi", [1, N], F32R, kind="ExternalInput")
    onesc = nc.dram_tensor("onesc", [128, 1], F32R, kind="ExternalInput")
    cmneg = nc.dram_tensor("cmneg", [1, HALF], F32R, kind="ExternalInput")
    out_d = nc.dram_tensor("out", [256, HALF], F32, kind="ExternalOutput")

    with tile.TileContext(nc, pool_alloc_mode="queue") as tc:
        with tc.tile_pool(name="persist", bufs=1) as persist:
            # G_aug [c, m]: rows 0..447 = Wg@skey+bg, row 448 (g[3][64]) = smi
            g = [
                persist.tile([128, N], F32R, tag=f"g{i}", name=f"g{i}")
                for i in range(3)
            ]
            g.append(persist.tile([65, N], F32R, tag="g3", name="g3"))
            ones_t = persist.tile([128, 1], F32R, tag="ones_t", name="ones_t")
            nc.sync.dma_start(ones_t, onesc[:, :])
            eps_t = persist.tile([128, 1], F32, tag="eps", name="eps")
            nc.vector.memset(eps_t, EPS)
            mgc = persist.tile([128, 512], mybir.dt.uint32, tag="mgc", name="mgc")
            nc.vector.memset(mgc, 0x5F3759DF)
            a_t = persist.tile([128, 2], F32, tag="a_t", name="a_t")
            b_t = persist.tile([128, 2], F32, tag="b_t", name="b_t")
            # ---- Projection phases, DMA-interleaved ----
            # B (G) runs first; C's (style) and B2's (ckey) input DMAs are
            # prefetched during earlier phases so each phase's first matmul
            # is never waiting on a cold queue.
            _pp = ExitStack()
            pc = _pp.enter_context(tc.tile_pool(name="projC", bufs=1))
            _pb_stack = ExitStack()
            pb = _pb_stack.enter_context(tc.tile_pool(name="projB", bufs=2))
            wgp = _pb_stack.enter_context(tc.tile_pool(name="wgp", bufs=1))
            ppb = _pb_stack.enter_context(
                tc.tile_pool(name="psumB", bufs=4, space="PSUM")
            )
            wg_t = wgp.tile([128, 4, 448], F32R, tag="wg_t", name="wg_t")
            nc.sync.dma_start(
                wg_t[:, 0:3, :], wgT[0:384, :].rearrange("(k p) c -> p k c", p=128)
            )
            nc.sync.dma_start(wg_t[0:65, 3, :], wgT[384:449, :])
            wh01 = pc.tile([128, 2, 256], F32R, tag="wh01", name="wh01")
            nc.sync.dma_start(
                wh01, whT[0:256, :].rearrange("(k p) c -> p k c", p=128)
            )
            bh1 = pc.tile([1, 256], F32, tag="bh1", name="bh1")
            nc.sync.dma_start(bh1, bhr[:, :])
            bhb = pc.tile([128, 256], F32, tag="bhb", name="bhb")
            nc.gpsimd.partition_broadcast(bhb, bh1)

            def load_st(sh):
                t = pc.tile([128, 2, N // 4], F32R, tag="st01", name="st01", bufs=2)
                ssl = slice(sh * (N // 4), (sh + 1) * (N // 4))
                nc.sync.dma_start(
                    t, styl[0:256, ssl].rearrange("(k p) m -> p k m", p=128)
                )
                return t

            st_pre = {}
            # Phase B: G
            for ch in range(8):
                sk = pb.tile([128, 4, 512], F32R, tag="sk", name="sk")
                csl = slice(ch * 512, (ch + 1) * 512)
                nc.sync.dma_start(
                    sk[:, 0:3, :],
                    skey[0:384, csl].rearrange("(k p) m -> p k m", p=128),
                )
                nc.sync.dma_start(sk[0:65, 3, :], skey[384:449, csl])
                for co, (co0, cosz) in enumerate(CO448):
                    pg = ppb.tile([128, 512], F32, tag="pg", name="pg")
                    for k, (k0, ksz) in enumerate(KT449):
                        nc.tensor.matmul(
                            pg[0:cosz, :],
                            lhsT=wg_t[0:ksz, k, co0 : co0 + cosz],
                            rhs=sk[0:ksz, k, :],
                            start=(k == 0),
                            stop=(k == 3),
                        )
                    dst = g[co][0:cosz, csl]
                    if co % 2 == 0:
                        nc.scalar.copy(dst, pg[0:cosz, :])
                    else:
                        nc.vector.tensor_copy(dst, pg[0:cosz, :])
                if ch == 5:
                    st_pre[0] = load_st(0)
                if ch == 7:
                    st_pre[1] = load_st(1)
            nc.sync.dma_start(g[3][64:65, :], smi[:, :])
            _pb_stack.close()

            # Phase C: Hv2 (wf/ck prefetched inside)
            pf_ = _pp.enter_context(tc.tile_pool(name="projF", bufs=2))
            wf_t = pf_.tile([128, 4, 448], F32R, tag="wf_t", name="wf_t", bufs=1)
            nc.sync.dma_start(
                wf_t[:, 0:3, :], wfT[0:384, :].rearrange("(k p) c -> p k c", p=128)
            )
            nc.sync.dma_start(wf_t[0:65, 3, :], wfT[384:449, :])

            def load_ck(ch):
                t = pf_.tile([128, 4, 512], F32R, tag="ck", name="ck", bufs=2)
                csl = slice(ch * 512, (ch + 1) * 512)
                nc.sync.dma_start(
                    t[:, 0:3, :],
                    ckey[0:384, csl].rearrange("(k p) m -> p k m", p=128),
                )
                nc.sync.dma_start(t[0:65, 3, :], ckey[384:449, csl])
                return t

            ck_pre = {}
            hv2 = persist.tile([128, 32, 512], F32R, tag="hv2", name="hv2")
            with tc.tile_pool(name="psumC", bufs=4, space="PSUM") as ppc:
                for sh in range(4):
                    st01 = st_pre.pop(sh) if sh in st_pre else load_st(sh)
                    for mt in range(sh * 8, sh * 8 + 8):
                        ph = ppc.tile([128, 256], F32, tag="ph", name="ph")
                        msl = slice(
                            mt * 128 - sh * (N // 4),
                            (mt + 1) * 128 - sh * (N // 4),
                        )
                        for k in range(2):
                            nc.tensor.matmul(
                                ph,
                                lhsT=st01[:, k, msl],
                                rhs=wh01[:, k, :],
                                start=(k == 0),
                                stop=(k == 1),
                            )
                        nc.vector.tensor_add(hv2[:, mt, 0:256], ph, bhb)
                        nc.vector.tensor_mul(
                            hv2[:, mt, 256:512],
                            hv2[:, mt, 0:256],
                            hv2[:, mt, 0:256],
                        )
                    if sh == 2:
                        ck_pre[0] = load_ck(0)
                    if sh == 3:
                        ck_pre[1] = load_ck(1)

            # Phase B2: Fq
            fq = [
                persist.tile([128, HALF], F32R, tag=f"fq{i}", name=f"fq{i}")
                for i in range(3)
            ]
            fq.append(persist.tile([65, HALF], F32R, tag="fq3", name="fq3"))
            with tc.tile_pool(name="psumF", bufs=4, space="PSUM") as ppf:
                for ch in range(4):
                    ck = ck_pre.pop(ch) if ch in ck_pre else load_ck(ch)
                    csl = slice(ch * 512, (ch + 1) * 512)
                    for co, (co0, cosz) in enumerate(CO448):
                        pq = ppf.tile([128, 512], F32, tag="pq", name="pq")
                        for k, (k0, ksz) in enumerate(KT449):
                            nc.tensor.matmul(
                                pq[0:cosz, :],
                                lhsT=wf_t[0:ksz, k, co0 : co0 + cosz],
                                rhs=ck[0:ksz, k, :],
                                start=(k == 0),
                                stop=(k == 3),
                            )
                        dst = fq[co][0:cosz, csl]
                        if co % 2 == 0:
                            nc.scalar.copy(dst, pq[0:cosz, :])
                        else:
                            nc.vector.tensor_copy(dst, pq[0:cosz, :])
            nc.sync.dma_start(fq[3][64:65, :], cmneg[:, :])
            _pp.close()

            # ---- Phase A: mvn stats over full content ----
            with tc.tile_pool(name="mvn", bufs=2) as pm:
                mvs = []
                for ct in range(2):
                    stats = pm.tile([128, 8, 6], F32, tag="stats", name="stats")
                    for half in range(2):
                        cx = pm.tile([128, N // 2], F32, tag="cx", name="cx")
                        nc.sync.dma_start(
                            cx,
                            cont[
                                ct * 128 : (ct + 1) * 128,
                                half * (N // 2) : (half + 1) * (N // 2),
                            ],
                        )
                        for i in range(4):
                            nc.vector.bn_stats(
                                out=stats[:, half * 4 + i, :],
                                in_=cx[:, i * 512 : (i + 1) * 512],
                            )
                    mv = pm.tile([128, 2], F32, tag="mv", name="mv", bufs=2)
                    nc.vector.bn_aggr(out=mv, in_=stats)
                    mvs.append(mv)
                lnvs = []
                for ct in range(2):
                    lnv = pm.tile([128, 1], F32, tag="lnv", name="lnv", bufs=2)
                    nc.scalar.activation(
                        lnv, mvs[ct][:, 1:2], AF.Ln, bias=eps_t[:, 0:1], scale=CORR
                    )
                    lnvs.append(lnv)
                for ct in range(2):
                    nc.scalar.activation(
                        a_t[:, ct : ct + 1], lnvs[ct], AF.Exp, scale=-0.5
                    )
                    nc.vector.scalar_tensor_tensor(
                        out=b_t[:, ct : ct + 1],
                        in0=mvs[ct][:, 0:1],
                        scalar=-1.0,
                        in1=a_t[:, ct : ct + 1],
                        op0=ALU.mult,
                        op1=ALU.mult,
                    )

            _dpools = ExitStack()
            fin = _dpools.enter_context(tc.tile_pool(name="fin", bufs=1))
            ptp = _dpools.enter_context(tc.tile_pool(name="ptp", bufs=1))

            # ---- Phase D: attention, transposed layout ----
            # T = S^T [m, n]; PT = exp(T); out[c,n] = Hv2^T @ PT directly in
            # output layout. No PE transposes anywhere.
            with (
                tc.tile_pool(name="ppt", bufs=2, space="PSUM") as ppt,
                tc.tile_pool(name="ppacc", bufs=1, space="PSUM") as ppacc,
            ):
                for ch in range(4):  # n-chunks of 512
                    nsl = slice(ch * 512, (ch + 1) * 512)
                    acc = ppacc.tile([128, 4, 512], F32, tag="acc", name="acc")
                    racc = ppacc.tile([1, 512], F32, tag="racc", name="racc")
                    # software-pipelined: MM2 lags one m-tile behind T/exp so
                    # the PE never waits on the just-issued exp.
                    def mm2(mt, pt_ap):
                        for c in range(4):
                            nc.tensor.matmul(
                                acc[:, c, :],
                                lhsT=hv2[:, mt, c * 128 : (c + 1) * 128],
                                rhs=pt_ap,
                                start=(mt == 0),
                                stop=(mt == 31),
                            )
                        nc.tensor.matmul(
                            racc,
                            lhsT=ones_t,
                            rhs=pt_ap,
                            start=(mt == 0),
                            stop=(mt == 31),
                        )

                    prev = None
                    for mh in range(8):  # m strips of 4 m-tiles
                        pt = ptp.tile(
                            [128, 4, 512], F32R, tag="pt", name="pt", bufs=2
                        )
                        for mi in range(4):
                            mt = mh * 4 + mi
                            msl = slice(mt * 128, (mt + 1) * 128)
                            tp = ppt.tile([128, 512], F32, tag="tp", name="tp")
                            for k, (k0, ksz) in enumerate(KT449):
                                nc.tensor.matmul(
                                    tp,
                                    lhsT=g[k][0:ksz, msl],
                                    rhs=fq[k][0:ksz, nsl],
                                    start=(k == 0),
                                    stop=(k == 3),
                                )
                            nc.scalar.activation(pt[:, mi, :], tp, AF.Exp)
                            if prev is not None:
                                mm2(*prev)
                            prev = (mt, pt[:, mi, :])
                    mm2(*prev)
                    # ---- finalize chunk (layout [c, n]) ----
                    # out = rinv*(sqrt(relu(m2*R - mean^2))*mvnc + mean_raw).
                    # ACT-free (Newton rsqrt + DVE reciprocal) so the exp
                    # table stays resident; acc released by one big copy.
                    accM = fin.tile(
                        [128, 4, 512], F32, tag="accM", name="accM", bufs=1
                    )
                    nc.vector.tensor_copy(accM, acc)
                    rsb = fin.tile([1, 512], F32, tag="fw", name="rsb", bufs=6)
                    nc.vector.tensor_copy(rsb, racc)
                    mm_ = [(accM[:, ct, :], accM[:, 2 + ct, :]) for ct in range(2)]
                    Rb = fin.tile([128, 512], F32, tag="mM", name="Rb", bufs=4)
                    nc.gpsimd.partition_broadcast(Rb, rsb)
                    rinv1 = fin.tile([1, 512], F32, tag="fw", name="rinv1", bufs=6)
                    nc.vector.reciprocal(rinv1, rsb)
                    rb = fin.tile([128, 512], F32, tag="mM", name="rb", bufs=4)
                    nc.gpsimd.partition_broadcast(rb, rinv1)
                    stds = []
                    for ct in range(2):
                        meanM, m2M = mm_[ct]
                        msq = fin.tile([128, 512], F32, tag="fw", name="msq", bufs=6)
                        nc.vector.tensor_mul(msq, meanM, meanM)
                        m2R = fin.tile([128, 512], F32, tag="fw", name="m2R", bufs=6)
                        nc.vector.tensor_mul(m2R, m2M, Rb)
                        varR = fin.tile([128, 512], F32, tag="fw", name="varR", bufs=6)
                        nc.vector.scalar_tensor_tensor(
                            out=varR,
                            in0=msq,
                            scalar=-1.0,
                            in1=m2R,
                            op0=ALU.mult,
                            op1=ALU.add,
                        )
                        varp = fin.tile([128, 512], F32, tag="fw", name="varp", bufs=6)
                        nc.vector.tensor_scalar_max(varp, varR, 0.0)
                        sh = fin.tile(
                            [128, 512], mybir.dt.uint32, tag="fw", name="sh", bufs=6
                        )
                        nc.vector.tensor_scalar(
                            sh,
                            varp.bitcast(mybir.dt.uint32),
                            1,
                            None,
                            ALU.logical_shift_right,
                        )
                        y = fin.tile([128, 512], F32, tag="fw", name="y0", bufs=6)
                        nc.vector.tensor_tensor(
                            out=y.bitcast(mybir.dt.uint32),
                            in0=mgc,
                            in1=sh,
                            op=ALU.subtract,
                        )
                        for it in range(2):
                            ta = fin.tile(
                                [128, 512], F32, tag="fw", name=f"ta{it}", bufs=6
                            )
                            nc.vector.tensor_mul(ta, y, y)
                            tb = fin.tile(
                                [128, 512], F32, tag="fw", name=f"tb{it}", bufs=6
                            )
                            nc.vector.tensor_mul(tb, ta, varp)
                            tcn = fin.tile(
                                [128, 512], F32, tag="fw", name=f"tc{it}", bufs=6
                            )
                            nc.vector.tensor_scalar(
                                tcn, tb, -0.5, 1.5, ALU.mult, ALU.add
                            )
                            y2 = fin.tile(
                                [128, 512], F32, tag="fw", name=f"y{it+1}", bufs=6
                            )
                            nc.vector.tensor_mul(y2, y, tcn)
                            y = y2
                        stdv = fin.tile([128, 512], F32, tag="fw", name="stdv", bufs=6)
                        nc.vector.tensor_mul(stdv, varp, y)
                        stds.append(stdv)
                    for ct in range(2):
                        meanM, _ = mm_[ct]
                        stdv = stds[ct]
                        csl2 = slice(ct * 128, (ct + 1) * 128)
                        cb = fin.tile([128, 512], F32, tag="fw", name="cb", bufs=6)
                        nc.sync.dma_start(cb, cont[csl2, nsl])
                        mvn_t = fin.tile([128, 512], F32, tag="fw", name="mvn", bufs=6)
                        nc.vector.tensor_scalar(
                            mvn_t,
                            cb,
                            a_t[:, ct : ct + 1],
                            b_t[:, ct : ct + 1],
                            ALU.mult,
                            ALU.add,
                        )
                        t1 = fin.tile([128, 512], F32, tag="fw", name="t1", bufs=6)
                        nc.vector.tensor_mul(t1, mvn_t, stdv)
                        t2 = fin.tile([128, 512], F32, tag="fw", name="t2", bufs=6)
                        nc.vector.tensor_add(t2, t1, meanM)
                        ob = fin.tile([128, 512], F32, tag="fw", name="ob", bufs=6)
                        nc.vector.tensor_mul(ob, t2, rb)
                        nc.sync.dma_start(out_d[csl2, nsl], ob)
            _dpools.close()
    nc.finalize()
    return nc


_nc_cache = None
last_results = None  # BassKernelResults of the most recent run (for test.py)


def prepare_in_maps(
    content,
    style,
    content_key,
    style_key,
    content_mask,
    style_mask,
    Wf,
    bf,
    Wg,
    bg,
    Wh,
    bh,
):
    f32 = np.float32
    ones_n = np.ones((1, N), f32)
    ones_h = np.ones((1, HALF), f32)
    wgT_in = np.ascontiguousarray(
        np.concatenate([np.asarray(Wg, f32).T, np.asarray(bg, f32)[None, :]], 0)
    )
    wfT_in = np.ascontiguousarray(
        np.concatenate([np.asarray(Wf, f32).T, np.asarray(bf, f32)[None, :]], 0)
    )
    whT_in = np.ascontiguousarray(np.asarray(Wh, f32).T)

    in_maps = []
    for c in range(8):
        b, h = divmod(c, 2)
        hsl = slice(h * HALF, (h + 1) * HALF)
        sk = np.asarray(style_key[b], f32).reshape(C_KEY, N)
        ck = np.asarray(content_key[b], f32).reshape(C_KEY, N)[:, hsl]
        st = np.asarray(style[b], f32).reshape(C_IN, N)
        co = np.asarray(content[b], f32).reshape(C_IN, N)
        smi_in = (np.asarray(content_mask, np.int32) * 0).astype(f32)  # placeholder
        smi_in = (np.asarray(style_mask[b], np.int32).reshape(1, N) == 0).astype(f32)
        cm = np.asarray(content_mask[b], np.int32).reshape(N)[hsl]
        cmneg_in = ((cm != 0).astype(f32) * np.float32(NEG))[None, :]
        in_maps.append(
            {
                "skey": np.ascontiguousarray(np.concatenate([sk, ones_n], 0)),
                "wgT": wgT_in,
                "ckey": np.ascontiguousarray(np.concatenate([ck, ones_h], 0)),
                "wfT": wfT_in,
                "styl": np.ascontiguousarray(st),
                "whT": whT_in,
                "bhr": np.ascontiguousarray(np.asarray(bh, f32)[None, :]),
                "cont": np.ascontiguousarray(
                    np.concatenate(
                        [co[:, hsl], co[:, (1 - h) * HALF : (2 - h) * HALF]], 1
                    )
                ),
                "smi": np.ascontiguousarray(smi_in),
                "onesc": np.ones((128, 1), dtype=f32),
                "cmneg": np.ascontiguousarray(cmneg_in),
            }
        )

    return in_maps


def get_nc():
    global _nc_cache
    if _nc_cache is None:
        _nc_cache = _build()
    return _nc_cache


def gather_output(outs):
    full = np.empty((B, C_IN, N), np.float32)
    for c in range(8):
        b, h = divmod(c, 2)
        full[b][:, h * HALF : (h + 1) * HALF] = outs[c]
    return full.reshape(B, C_IN, 64, 64)


def kernel(**inputs):
    global last_results
    in_maps = prepare_in_maps(**inputs)
    res = run_bass_kernel_spmd(get_nc(), in_maps, core_ids=list(range(8)))
    last_results = res
    return gather_output([r["out"] for r in res.results])


if __name__ == "__main__":
    rng = np.random.default_rng(0)
    ins = {
        "content": rng.standard_normal((B, C_IN, 64, 64), dtype=np.float32),
        "style": rng.standard_normal((B, C_IN, 64, 64), dtype=np.float32),
        "content_key": rng.standard_normal((B, C_KEY, 64, 64), dtype=np.float32),
        "style_key": rng.standard_normal((B, C_KEY, 64, 64), dtype=np.float32),
        "content_mask": rng.integers(0, 2, (B, 1, 64, 64)).astype(np.int32),
        "style_mask": rng.integers(0, 2, (B, 1, 64, 64)).astype(np.int32),
        "Wf": (rng.standard_normal((C_KEY, C_KEY)) * 0.02).astype(np.float32),
        "bf": (rng.standard_normal((C_KEY,)) * 0.02).astype(np.float32),
        "Wg": (rng.standard_normal((C_KEY, C_KEY)) * 0.02).astype(np.float32),
        "bg": (rng.standard_normal((C_KEY,)) * 0.02).astype(np.float32),
        "Wh": (rng.standard_normal((C_IN, C_IN)) * 0.02).astype(np.float32),
        "bh": (rng.standard_normal((C_IN,)) * 0.02).astype(np.float32),
    }
    out = kernel(**ins)
    print("kernel output", out.shape, out.dtype, np.abs(out).mean())



# revision 10
# speedup vs baseline: 1.2923x; 1.2923x over previous
"""Trainium2 Bass kernel for the masked style-attention module.

Shapes (hardcoded): B=4, C_IN=256, C_KEY=448, H=W=64, N=4096.
Sharding: 8 cores = batch (4) x query-column half (2). Each core computes
out[b][:, h*2048:(h+1)*2048] for its (b, h).

Key restructurings vs the v1 kernel:
- K-fold: S = (Wf ckey+bf)^T (Wg skey+bg) = ckey_aug^T M skey_aug with
  M = Wf_aug @ Wg_aug^T computed on host (weights-only preprocessing).
  The entire G projection phase disappears; the device only computes
  fqA = M^T @ ckey_aug ([449, 2048]) and uses skey_aug directly as the
  stationary operand of the S^T matmul.
- Mask folded as contraction row 449: skeyA row449 = (style_mask==0),
  fqA row449 = NEG*(content_mask!=0).  Bias rows via ones rows (448).
- Phase D (attention) starts as soon as fqA chunk 0 is projected; the
  Hv2 projection, mvn stats and fq projections for chunks 1..3 are all
  interleaved into the chunk loops, overlapping the input DMA stream.
- Stationary matmul operands (skeyA, Hv2) are bf16: halves LDWEIGHTS
  traffic + DMA + SBUF; moving operands stay f32r (full rate at free>=256).
- Rowsum computed broadcast into all 128 PSUM partitions (all-ones
  [128,128] stationary tile) so no gpsimd partition_broadcast is needed.
- Finalize uses ACT Square/Sqrt (1 table switch per chunk) instead of a
  long Newton-rsqrt DVE chain; last chunk reads acc from PSUM directly.

Math per core (rows m in 0..4095, cols n in its half):
  fqA[k,n] = (M^T ckey_aug)[k,n]; row449 = NEG*cm[n]
  T[m,n] = sum_k skeyA[k,m] fqA[k,n]   (= S^T, mask/bias folded in)
  P = exp(T)  (|T| < ~40: no overflow; softmax shift-invariant)
  acc[c,n] = sum_m Hv2[m,c] P[m,n],  racc[n] = sum_m P[m,n]  (Hv2=[Hv,Hv^2])
  u = 1/racc; mean = acc[0:256]*u; m2 = acc[256:512]*u
  out[c,n] = sqrt(relu(m2-mean^2))*(a_c*cont+b_c) + mean
"""

import numpy as np
import ml_dtypes

import concourse.bass as bass
from concourse import bacc
import concourse.mybir as mybir
import concourse.tile as tile
from concourse.bass_utils import run_bass_kernel_spmd

AF = mybir.ActivationFunctionType
ALU = mybir.AluOpType
F32 = mybir.dt.float32
F32R = mybir.dt.float32r
BF16 = mybir.dt.bfloat16

B, C_IN, C_KEY = 4, 256, 448
N = 4096
HALF = 2048
NEG = -1e15
EPS = 1e-5
CORR = N / (N - 1.0)  # unbiased-variance correction for mvn

# contraction tiles over 449 (= C_KEY + ones row) for the fq projection
KT449 = [(0, 128), (128, 128), (256, 128), (384, 65)]
# output-row tiles over 449 for fqA
CO449 = [(0, 128), (128, 128), (256, 128), (384, 65)]
# contraction tiles over 450 for T (449 + mask row): s3/fq3 are 66 rows
KS450 = [128, 128, 128, 66]


def _build():
    nc = bacc.Bacc("TRN2", target_bir_lowering=False)

    mT = nc.dram_tensor("mT", [449, 449], F32R, kind="ExternalInput")
    ckeyA = nc.dram_tensor("ckeyA", [449, HALF], F32R, kind="ExternalInput")
    cmneg = nc.dram_tensor("cmneg", [1, HALF], BF16, kind="ExternalInput")
    skeyA = nc.dram_tensor("skeyA", [450, N], BF16, kind="ExternalInput")
    styl = nc.dram_tensor("styl", [256, N], F32R, kind="ExternalInput")
    whA = nc.dram_tensor("whA", [257, 256], F32R, kind="ExternalInput")
    cont = nc.dram_tensor("cont", [256, N], F32, kind="ExternalInput")
    onesb_d = nc.dram_tensor("onesb", [128, 128], BF16, kind="ExternalInput")
    ones1_d = nc.dram_tensor("ones1", [1, 128], F32R, kind="ExternalInput")
    out_d = nc.dram_tensor("out", [256, HALF], F32, kind="ExternalOutput")

    with tile.TileContext(nc, pool_alloc_mode="queue") as tc:
        with (
            tc.tile_pool(name="persist", bufs=1) as persist,
            tc.tile_pool(name="stream", bufs=1) as stream,
            tc.tile_pool(name="fin", bufs=1) as fin,
            tc.tile_pool(name="pacc", bufs=1, space="PSUM") as pacc,
            tc.tile_pool(name="ppt", bufs=2, space="PSUM") as ppt,
            tc.tile_pool(name="pph", bufs=1, space="PSUM") as pph,
        ):
            # ---- persistent tiles ----
            s = [
                persist.tile([128, N], BF16, tag=f"s{i}", name=f"s{i}")
                for i in range(3)
            ]
            s.append(persist.tile([66, N], BF16, tag="s3", name="s3"))
            fq = [
                persist.tile([128, HALF], BF16, tag=f"fq{i}", name=f"fq{i}")
                for i in range(3)
            ]
            fq.append(persist.tile([66, HALF], BF16, tag="fq3", name="fq3"))
            hv2 = persist.tile([128, 32, 512], BF16, tag="hv2", name="hv2")
            mT_t = persist.tile([128, 4, 449], F32R, tag="mT_t", name="mT_t")
            whA_t = persist.tile([128, 2, 256], F32R, tag="whA_t", name="whA_t")
            whb = persist.tile([1, 256], F32R, tag="whb", name="whb")
            onesb = persist.tile([128, 128], BF16, tag="onesb", name="onesb")
            ones1 = persist.tile([1, 128], F32R, tag="ones1", name="ones1")
            eps_t = persist.tile([128, 1], F32, tag="eps", name="eps")
            a_t = persist.tile([128, 2], F32, tag="a_t", name="a_t")
            b_t = persist.tile([128, 2], F32, tag="b_t", name="b_t")
            stats = persist.tile([128, 2, 8, 6], F32, tag="stats", name="stats")
            mvs = persist.tile([128, 2, 2], F32, tag="mvs", name="mvs")

            nc.vector.memset(eps_t, EPS)

            # ---- priority DMAs: weights + chunk-0 inputs ----
            nc.sync.dma_start(
                mT_t[:, 0:3, :], mT[0:384, :].rearrange("(k p) c -> p k c", p=128)
            )
            nc.sync.dma_start(mT_t[0:65, 3, :], mT[384:449, :])
            nc.sync.dma_start(
                whA_t, whA[0:256, :].rearrange("(k p) c -> p k c", p=128)
            )
            nc.sync.dma_start(whb, whA[256:257, :])
            nc.sync.dma_start(onesb, onesb_d[:, :])
            nc.sync.dma_start(ones1, ones1_d[:, :])
            nc.sync.dma_start(fq[3][65:66, :], cmneg[:, :])

            def load_ck(ch):
                t = stream.tile([128, 4, 512], F32R, tag="ck", name="ck", bufs=2)
                csl = slice(ch * 512, (ch + 1) * 512)
                nc.sync.dma_start(
                    t[:, 0:3, :],
                    ckeyA[0:384, csl].rearrange("(k p) m -> p k m", p=128),
                )
                nc.sync.dma_start(t[0:65, 3, :], ckeyA[384:449, csl])
                return t

            def load_skey(mc):
                msl = slice(mc * 512, (mc + 1) * 512)
                for i in range(3):
                    nc.sync.dma_start(
                        s[i][:, msl], skeyA[i * 128 : (i + 1) * 128, msl]
                    )
                nc.sync.dma_start(s[3][:, msl], skeyA[384:450, msl])

            def load_st(mt):
                t = stream.tile([128, 2, 128], F32R, tag="st", name="st", bufs=8)
                msl = slice(mt * 128, (mt + 1) * 128)
                nc.sync.dma_start(
                    t, styl[0:256, msl].rearrange("(k p) m -> p k m", p=128)
                )
                return t

            ck0 = load_ck(0)
            ck_pre = load_ck(1)
            load_skey(0)
            st_tiles = {mt: load_st(mt) for mt in range(4)}

            # ---- fqA projection for chunk 0 (full 512-wide via tp ring) ----
            ck_cur = ck0
            for co, (co0, cosz) in enumerate(CO449):
                pq = ppt.tile([128, 512], F32, tag="tp", name="pq")
                for k, (k0, ksz) in enumerate(KT449):
                    nc.tensor.matmul(
                        pq[0:cosz, :],
                        lhsT=mT_t[0:ksz, k, co0 : co0 + cosz],
                        rhs=ck_cur[0:ksz, k, 0:512],
                        start=(k == 0),
                        stop=(k == 3),
                    )
                if co % 2 == 0:
                    nc.scalar.copy(fq[co][0:cosz, 0:512], pq[0:cosz, :])
                else:
                    nc.vector.tensor_copy(fq[co][0:cosz, 0:512], pq[0:cosz, :])

            # ---- helpers for interleaved side work ----
            ck_next = [None]

            def fq_proj_piece(ch, grp):
                # one (co, n-half) group of the fqA projection for chunk ch:
                # 4 matmuls (256-wide) + 1 copy, through the php psum ring
                co = grp // 2
                nh = grp % 2
                co0, cosz = CO449[co]
                nsl = slice(ch * 512 + nh * 256, ch * 512 + (nh + 1) * 256)
                rsl = slice(nh * 256, (nh + 1) * 256)
                pj = pph.tile([128, 256], F32, tag="php", name="pj")
                for k, (k0, ksz) in enumerate(KT449):
                    nc.tensor.matmul(
                        pj[0:cosz, :],
                        lhsT=mT_t[0:ksz, k, co0 : co0 + cosz],
                        rhs=ck_next[0][0:ksz, k, rsl],
                        start=(k == 0),
                        stop=(k == 3),
                    )
                if grp % 2 == 0:
                    nc.scalar.copy(fq[co][0:cosz, nsl], pj[0:cosz, :])
                else:
                    nc.vector.tensor_copy(fq[co][0:cosz, nsl], pj[0:cosz, :])

            cx_tiles = {}

            def load_cx(ct, half):
                t = stream.tile([128, HALF], F32, tag="cx", name="cx", bufs=2)
                nc.sync.dma_start(
                    t,
                    cont[
                        ct * 128 : (ct + 1) * 128,
                        half * HALF : (half + 1) * HALF,
                    ],
                )
                cx_tiles[(ct, half)] = t

            def stats_piece(ct, half):
                t = cx_tiles.pop((ct, half))
                for i in range(4):
                    nc.vector.bn_stats(
                        out=stats[:, ct, half * 4 + i, :],
                        in_=t[:, i * 512 : (i + 1) * 512],
                    )

            def mvn_aggr(ct):
                nc.vector.bn_aggr(out=mvs[:, ct, :], in_=stats[:, ct, :, :])

            def mvn_ab(ct):
                lnv = fin.tile([128, 1], F32, tag="lnv", name="lnv", bufs=2)
                nc.scalar.activation(
                    lnv, mvs[:, ct, 1:2], AF.Ln, bias=eps_t[:, 0:1], scale=CORR
                )
                nc.scalar.activation(
                    a_t[:, ct : ct + 1], lnv, AF.Exp, scale=-0.5
                )
                nc.vector.scalar_tensor_tensor(
                    out=b_t[:, ct : ct + 1],
                    in0=mvs[:, ct, 0:1],
                    scalar=-1.0,
                    in1=a_t[:, ct : ct + 1],
                    op0=ALU.mult,
                    op1=ALU.mult,
                )

            def side_work(ch, mh):
                if ch == 0:
                    if mh < 7:
                        load_skey(mh + 1)
                        for mt in range((mh + 1) * 4, (mh + 2) * 4):
                            st_tiles[mt] = load_st(mt)
                    if mh == 0:
                        ck_next[0] = ck_pre
                    if mh == 1:
                        load_cx(0, 0)
                    if mh == 2:
                        load_cx(0, 1)
                        stats_piece(0, 0)
                    if mh == 3:
                        stats_piece(0, 1)
                        load_cx(1, 0)
                    if mh == 4:
                        mvn_aggr(0)
                        load_cx(1, 1)
                        stats_piece(1, 0)
                    if mh == 5:
                        stats_piece(1, 1)
                    if mh == 6:
                        mvn_aggr(1)
                        mvn_ab(0)
                        mvn_ab(1)
                elif ch < 3 and mh == 0:
                    ck_next[0] = load_ck(ch + 1)
                if ch < 3:
                    fq_proj_piece(ch + 1, mh)

            def hv_proj(mt):
                st = st_tiles.pop(mt)
                ph = pph.tile([128, 256], F32, tag="php", name="ph")
                nc.tensor.matmul(
                    ph, lhsT=st[:, 0, :], rhs=whA_t[:, 0, :], start=True, stop=False
                )
                nc.tensor.matmul(
                    ph, lhsT=st[:, 1, :], rhs=whA_t[:, 1, :], start=False, stop=False
                )
                nc.tensor.matmul(
                    ph, lhsT=ones1[0:1, :], rhs=whb, start=False, stop=True
                )
                nc.vector.tensor_copy(hv2[:, mt, 0:256], ph)
                nc.scalar.activation(hv2[:, mt, 256:512], ph, AF.Square)

            # ---- phase D: 4 n-chunks of 512 ----
            for ch in range(4):
                nsl = slice(ch * 512, (ch + 1) * 512)
                acc = pacc.tile([128, 4, 512], F32, tag="acc", name="acc")
                racc = pacc.tile([128, 512], F32, tag="racc", name="racc")

                def mm2(mt, pt_ap):
                    for c in range(4):
                        nc.tensor.matmul(
                            acc[:, c, :],
                            lhsT=hv2[:, mt, c * 128 : (c + 1) * 128],
                            rhs=pt_ap,
                            start=(mt == 0),
                            stop=(mt == 31),
                        )
                    nc.tensor.matmul(
                        racc,
                        lhsT=onesb,
                        rhs=pt_ap,
                        start=(mt == 0),
                        stop=(mt == 31),
                    )

                prev = None
                for mh in range(8):
                    side_work(ch, mh)
                    pt = stream.tile(
                        [128, 4, 512], BF16, tag="pt", name="pt", bufs=3
                    )
                    for mi in range(4):
                        mt = mh * 4 + mi
                        if ch == 0:
                            hv_proj(mt)
                        msl = slice(mt * 128, (mt + 1) * 128)
                        tp = ppt.tile([128, 512], F32, tag="tp", name="tp")
                        for k in range(4):
                            ksz = KS450[k]
                            nc.tensor.matmul(
                                tp,
                                lhsT=s[k][0:ksz, msl],
                                rhs=fq[k][0:ksz, nsl],
                                start=(k == 0),
                                stop=(k == 3),
                            )
                        nc.scalar.activation(pt[:, mi, :], tp, AF.Exp)
                        if prev is not None:
                            mm2(*prev)
                        prev = (mt, pt[:, mi, :])
                mm2(*prev)

                # ---- finalize chunk (output layout [c, n]) ----
                # acc-freeing ops first (u, mb, m2b) so the next chunk's mm2
                # can reclaim the PSUM banks quickly.
                u = fin.tile([128, 512], F32, tag="u", name="u")
                nc.vector.reciprocal_approx_fast(u, racc)
                mbs, m2bs, cbs = [], [], []
                for ct in range(2):
                    mb = fin.tile([128, 512], F32, tag="mb", name="mb", bufs=2)
                    nc.vector.tensor_mul(mb, acc[:, ct, :], u)
                    mbs.append(mb)
                    m2b = fin.tile([128, 512], F32, tag="m2b", name="m2b", bufs=2)
                    nc.vector.tensor_mul(m2b, acc[:, 2 + ct, :], u)
                    m2bs.append(m2b)
                for ct in range(2):
                    cb = fin.tile([128, 512], F32, tag="cb", name="cb", bufs=2)
                    nc.sync.dma_start(
                        cb, cont[ct * 128 : (ct + 1) * 128, nsl]
                    )
                    cbs.append(cb)
                for ct in range(2):
                    msq = fin.tile([128, 512], F32, tag="msq", name="msq", bufs=2)
                    nc.scalar.activation(msq, mbs[ct], AF.Square)
                    varp = fin.tile([128, 512], F32, tag="varp", name="varp", bufs=2)
                    nc.vector.scalar_tensor_tensor(
                        out=varp,
                        in0=msq,
                        scalar=-1.0,
                        in1=m2bs[ct],
                        op0=ALU.mult,
                        op1=ALU.add,
                    )
                    varm = fin.tile([128, 512], F32, tag="varm", name="varm", bufs=2)
                    nc.vector.tensor_scalar_max(varm, varp, 0.0)
                    stdv = fin.tile([128, 512], F32, tag="stdv", name="stdv", bufs=2)
                    nc.scalar.activation(stdv, varm, AF.Sqrt)
                    mvnc = fin.tile([128, 512], F32, tag="mvnc", name="mvnc", bufs=2)
                    nc.vector.tensor_scalar(
                        mvnc,
                        cbs[ct],
                        a_t[:, ct : ct + 1],
                        b_t[:, ct : ct + 1],
                        ALU.mult,
                        ALU.add,
                    )
                    t1 = fin.tile([128, 512], F32, tag="t1", name="t1", bufs=2)
                    nc.vector.tensor_mul(t1, stdv, mvnc)
                    ob = fin.tile([128, 512], F32, tag="ob", name="ob", bufs=2)
                    nc.vector.tensor_add(ob, t1, mbs[ct])
                    nc.sync.dma_start(
                        out_d[ct * 128 : (ct + 1) * 128, nsl], ob
                    )
    nc.finalize()
    return nc


_nc_cache = None
last_results = None  # BassKernelResults of the most recent run (for test.py)


def prepare_in_maps(
    content,
    style,
    content_key,
    style_key,
    content_mask,
    style_mask,
    Wf,
    bf,
    Wg,
    bg,
    Wh,
    bh,
):
    f32 = np.float32
    bf16 = ml_dtypes.bfloat16
    ones_n = np.ones((1, N), f32)
    ones_h = np.ones((1, HALF), f32)

    Wf_aug = np.concatenate([np.asarray(Wf, f32).T, np.asarray(bf, f32)[None, :]], 0)
    Wg_aug = np.concatenate([np.asarray(Wg, f32).T, np.asarray(bg, f32)[None, :]], 0)
    mT_in = np.ascontiguousarray(Wf_aug @ Wg_aug.T)  # [449, 449]
    whA_in = np.ascontiguousarray(
        np.concatenate([np.asarray(Wh, f32).T, np.asarray(bh, f32)[None, :]], 0)
    )
    onesb_in = np.ones((128, 128), bf16)
    ones1_in = np.ones((1, 128), f32)

    in_maps = []
    for c in range(8):
        b, h = divmod(c, 2)
        hsl = slice(h * HALF, (h + 1) * HALF)
        sk = np.asarray(style_key[b], f32).reshape(C_KEY, N)
        ck = np.asarray(content_key[b], f32).reshape(C_KEY, N)[:, hsl]
        st = np.asarray(style[b], f32).reshape(C_IN, N)
        co = np.asarray(content[b], f32).reshape(C_IN, N)
        smi = (np.asarray(style_mask[b], np.int32).reshape(1, N) == 0).astype(f32)
        cm = np.asarray(content_mask[b], np.int32).reshape(N)[hsl]
        cmneg_in = ((cm != 0).astype(f32) * np.float32(NEG))[None, :].astype(bf16)
        skA = np.concatenate([sk, ones_n, smi], 0).astype(bf16)
        in_maps.append(
            {
                "mT": mT_in,
                "ckeyA": np.ascontiguousarray(np.concatenate([ck, ones_h], 0)),
                "cmneg": np.ascontiguousarray(cmneg_in),
                "skeyA": np.ascontiguousarray(skA),
                "styl": np.ascontiguousarray(st),
                "whA": whA_in,
                "cont": np.ascontiguousarray(
                    np.concatenate(
                        [co[:, hsl], co[:, (1 - h) * HALF : (2 - h) * HALF]], 1
                    )
                ),
                "onesb": onesb_in,
                "ones1": ones1_in,
            }
        )
    return in_maps


def get_nc():
    global _nc_cache
    if _nc_cache is None:
        _nc_cache = _build()
    return _nc_cache


def gather_output(outs):
    full = np.empty((B, C_IN, N), np.float32)
    for c in range(8):
        b, h = divmod(c, 2)
        full[b][:, h * HALF : (h + 1) * HALF] = outs[c]
    return full.reshape(B, C_IN, 64, 64)


def kernel(**inputs):
    global last_results
    in_maps = prepare_in_maps(**inputs)
    res = run_bass_kernel_spmd(get_nc(), in_maps, core_ids=list(range(8)))
    last_results = res
    return gather_output([r["out"] for r in res.results])


if __name__ == "__main__":
    rng = np.random.default_rng(0)
    ins = {
        "content": rng.standard_normal((B, C_IN, 64, 64), dtype=np.float32),
        "style": rng.standard_normal((B, C_IN, 64, 64), dtype=np.float32),
        "content_key": rng.standard_normal((B, C_KEY, 64, 64), dtype=np.float32),
        "style_key": rng.standard_normal((B, C_KEY, 64, 64), dtype=np.float32),
        "content_mask": rng.integers(0, 2, (B, 1, 64, 64)).astype(np.int32),
        "style_mask": rng.integers(0, 2, (B, 1, 64, 64)).astype(np.int32),
        "Wf": (rng.standard_normal((C_KEY, C_KEY)) * 0.02).astype(np.float32),
        "bf": (rng.standard_normal((C_KEY,)) * 0.02).astype(np.float32),
        "Wg": (rng.standard_normal((C_KEY, C_KEY)) * 0.02).astype(np.float32),
        "bg": (rng.standard_normal((C_KEY,)) * 0.02).astype(np.float32),
        "Wh": (rng.standard_normal((C_IN, C_IN)) * 0.02).astype(np.float32),
        "bh": (rng.standard_normal((C_IN,)) * 0.02).astype(np.float32),
    }
    out = kernel(**ins)
    print("kernel output", out.shape, out.dtype, np.abs(out).mean())
